# revision 4
# baseline (speedup 1.0000x reference)
"""Masked multi-head self-attention (sparse_attention) on 8 Trainium2 cores.

Strategy
--------
Shard the fused (batch*heads)=16 leading dim of q/k/v across 8 cores, 2 heads
per core.  Per head the kernel computes S^T = K @ Q^T in [j, i] orientation
(128-row j-chunks on partitions, 512-col i-blocks on the free dim), applies
exp on the scalar engine (no max-subtraction needed: |s*scale| <= ~7 so exp
cannot overflow in fp32, and blocked entries are handled structurally, not
additively), then accumulates O^T = V~^T @ P^T on the tensor engine where
V~ = [V | 1] so the softmax denominators fall out of the same matmuls.

The bbox mask has rank-structure: blocked(i,j) <=> (i in A-only and j in
B-only) or vice versa, where A/B are the two subject boxes.  The host sorts
the j (key/value) axis into [A-only | B-only | rest] with 64-aligned zero
padding, so every 64-row half-chunk belongs to one group.  PV matmuls
accumulate into one PSUM accumulator per group; the final combine applies the
per-i 0/1 weights (wA, wB) and sums the three accumulators — the mask costs
no elementwise work on the n*n tiles at all.  Finally each [81, 512] combined
block is PE-transposed back to [128(i), 81], normalized by the sums column,
and DMA'd out in natural i order (the i/query axis is never permuted).
"""

import math
import os

import numpy as np

N_CORES = 8
P = 128  # partitions / j-chunk rows
IB = 512  # i-block width (psum bank, fp32)
DH = 80  # head dim
DV = DH + 1  # V plus the ones column

_PROGRAM_CACHE = {}
LAST_RESULTS = None  # BassKernelResults of the most recent run (for test.py)


# ----------------------------------------------------------------------------
# host-side mask analysis (mirrors reference._subject_masks / _self_mask)
# ----------------------------------------------------------------------------

def _subject_masks_np(bboxes: np.ndarray, resolution: int) -> np.ndarray:
    b = bboxes[0].astype(np.float32)  # [s, 4]
    x0 = np.round(b[:, 0] * resolution)
    y0 = np.round(b[:, 1] * resolution)
    x1 = np.round(b[:, 2] * resolution)
    y1 = np.round(b[:, 3] * resolution)
    coords = np.arange(resolution, dtype=np.float32)
    xm = (coords[None, :] >= x0[:, None]) & (coords[None, :] < x1[:, None])
    ym = (coords[None, :] >= y0[:, None]) & (coords[None, :] < y1[:, None])
    return (ym[:, :, None] & xm[:, None, :]).reshape(b.shape[0], -1)  # [s, n]


def _group_layout(bboxes: np.ndarray, n: int):
    """Sort the j axis into [A-only | B-only | rest], 64-aligned groups.

    Returns (perm, seg_sizes, group_starts, n_pad, wA, wB) where perm is the
    source index for each padded slot (-1 for zero padding), wA/wB are the
    per-original-i {0,1} combine weights for the A/B accumulators.
    """
    res = int(math.isqrt(n))
    assert res * res == n
    subj = _subject_masks_np(bboxes, res)
    assert subj.shape[0] == 2, "kernel specialized for 2 subject boxes"
    m0, m1 = subj[0], subj[1]
    e0 = m0 & ~m1  # A-only
    e1 = m1 & ~m0  # B-only
    rest = ~(e0 | e1)

    idx = np.arange(n)
    groups = [idx[e0], idx[e1], idx[rest]]

    def ceil64(x):
        return ((x + 63) // 64) * 64

    padded = [ceil64(len(g)) for g in groups]
    n_pad = sum(padded)
    if n_pad % P:
        padded[2] += 64
        n_pad += 64
    perm = np.full(n_pad, -1, dtype=np.int64)
    starts = []
    pos = 0
    for g, plen in zip(groups, padded):
        starts.append(pos)
        perm[pos : pos + len(g)] = g
        pos += plen

    wA = (~e1).astype(np.float32)  # zero the A accumulator for i in B-only
    wB = (~e0).astype(np.float32)  # zero the B accumulator for i in A-only
    return perm, padded, starts, n_pad, wA, wB


def _chunk_segments(padded, starts, n_pad):
    """Per 128-chunk: list of (row_lo, row_hi, group_id) 64-aligned segments."""
    half_group = np.empty(n_pad // 64, dtype=np.int64)
    for gid, (st, plen) in enumerate(zip(starts, padded)):
        half_group[st // 64 : (st + plen) // 64] = gid
    segments = []
    for c in range(n_pad // P):
        g0 = int(half_group[2 * c])
        g1 = int(half_group[2 * c + 1])
        if g0 == g1:
            segments.append([(0, P, g0)])
        else:
            segments.append([(0, 64, g0), (64, P, g1)])
    return segments


# ----------------------------------------------------------------------------
# device program
# ----------------------------------------------------------------------------

def _build_program(n, n_pad, heads_per_core, segments, present_groups, scale):
    import concourse.mybir as mybir
    import concourse.tile as tile
    from concourse import bacc

    f32 = mybir.dt.float32
    nch = n_pad // P
    n_ib = n // IB
    Exp = mybir.ActivationFunctionType.Exp
    MUL = mybir.AluOpType.mult
    ADD = mybir.AluOpType.add

    nc = bacc.Bacc("TRN2", target_bir_lowering=False, debug=False,
                   num_devices=N_CORES)
    qT_d = nc.dram_tensor("qT", [heads_per_core, DH, n], f32, kind="ExternalInput")
    kT_d = nc.dram_tensor("kT", [heads_per_core, DH, n_pad], f32, kind="ExternalInput")
    vt_d = nc.dram_tensor("vt", [heads_per_core, n_pad, DV], f32, kind="ExternalInput")
    wab_d = nc.dram_tensor("wab", [DV, n], f32, kind="ExternalInput")
    wbb_d = nc.dram_tensor("wbb", [DV, n], f32, kind="ExternalInput")
    id_d = nc.dram_tensor("ident", [P, P], f32, kind="ExternalInput")
    o_d = nc.dram_tensor("o", [heads_per_core, n, DH], f32, kind="ExternalOutput")

    # chunk pairs for batched exp reads
    pairs = [tuple(range(t, min(t + 2, nch))) for t in range(0, nch, 2)]

    # first/last (chunk, row) PV matmul per accumulator group, for start/stop
    first_seg = {}
    last_seg = {}
    for c, segs in enumerate(segments):
        for (r0, _, g) in segs:
            first_seg.setdefault(g, (c, r0))
            last_seg[g] = (c, r0)

    with tile.TileContext(nc) as tc:
        with (
            tc.tile_pool(name="const", bufs=1) as const_pool,
            tc.tile_pool(name="head", bufs=2) as head_pool,
            tc.tile_pool(name="p", bufs=3) as p_pool,
            tc.tile_pool(name="comb", bufs=2) as comb_pool,
            tc.tile_pool(name="out", bufs=4) as out_pool,
            tc.tile_pool(name="s_ps", bufs=2, space="PSUM") as s_pool,
            tc.tile_pool(name="acc_ps", bufs=1, space="PSUM") as acc_pool,
            tc.tile_pool(name="tr_ps", bufs=1, space="PSUM") as tr_pool,
        ):
            ident = const_pool.tile([P, P], f32)
            nc.sync.dma_start(ident[:], id_d[:])
            wab_t = const_pool.tile([DV, n], f32)
            nc.sync.dma_start(wab_t[:], wab_d[:])
            wbb_t = const_pool.tile([DV, n], f32)
            nc.sync.dma_start(wbb_t[:], wbb_d[:])

            for h in range(heads_per_core):
                kT_t = head_pool.tile([DH, nch, P], f32, tag="kT")
                nc.sync.dma_start(
                    kT_t[:], kT_d[h].rearrange("d (c j) -> d c j", j=P)
                )
                qT_t = head_pool.tile([DH, n], f32, tag="qT")
                nc.sync.dma_start(qT_t[:], qT_d[h])
                vt_t = head_pool.tile([P, nch, DV], f32, tag="vt")
                nc.sync.dma_start(
                    vt_t[:], vt_d[h].rearrange("(c p) d -> p c d", p=P)
                )

                for ib in range(n_ib):
                    accs = {
                        g: acc_pool.tile([DV, IB], f32, tag=f"acc{g}",
                                         name=f"acc{g}_{h}_{ib}")
                        for g in present_groups
                    }
                    q_sl = qT_t[:, ib * IB : (ib + 1) * IB]
                    for pr in pairs:
                        s_t = s_pool.tile([P, IB * len(pr)], f32, tag="s")
                        for pi, c in enumerate(pr):
                            nc.tensor.matmul(
                                s_t[:, pi * IB : (pi + 1) * IB],
                                lhsT=kT_t[:, c, :],
                                rhs=q_sl,
                                start=True,
                                stop=True,
                            )
                        p_t = p_pool.tile([P, IB * len(pr)], f32, tag="p")
                        nc.scalar.activation(p_t[:], s_t[:], Exp, scale=scale)
                        for pi, c in enumerate(pr):
                            for (r0, r1, g) in segments[c]:
                                nc.tensor.matmul(
                                    accs[g][:],
                                    lhsT=vt_t[r0:r1, c, :],
                                    rhs=p_t[r0:r1, pi * IB : (pi + 1) * IB],
                                    start=((c, r0) == first_seg[g]),
                                    stop=((c, r0) == last_seg[g]),
                                )

                    # combine accumulators -> [DV, IB] in SBUF
                    comb = comb_pool.tile([DV, IB], f32, tag="comb")
                    i_sl = slice(ib * IB, (ib + 1) * IB)
                    if 0 in accs and 1 in accs:
                        t1 = comb_pool.tile([DV, IB], f32, tag="t1")
                        nc.vector.tensor_tensor(
                            t1[:], accs[0][:], wab_t[:, i_sl], op=MUL
                        )
                        t2 = comb_pool.tile([DV, IB], f32, tag="t2")
                        nc.vector.tensor_tensor(
                            t2[:], accs[1][:], wbb_t[:, i_sl], op=MUL
                        )
                        nc.vector.tensor_tensor(t1[:], t1[:], t2[:], op=ADD)
                        nc.vector.tensor_tensor(comb[:], t1[:], accs[2][:], op=ADD)
                    elif 0 in accs or 1 in accs:
                        g = 0 if 0 in accs else 1
                        w_t = wab_t if g == 0 else wbb_t
                        t1 = comb_pool.tile([DV, IB], f32, tag="t1")
                        nc.vector.tensor_tensor(
                            t1[:], accs[g][:], w_t[:, i_sl], op=MUL
                        )
                        nc.vector.tensor_tensor(comb[:], t1[:], accs[2][:], op=ADD)
                    else:
                        nc.vector.tensor_copy(comb[:], accs[2][:])

                    # transpose quarters, normalize, store
                    for qq in range(IB // P):
                        tr = tr_pool.tile([P, DV], f32, tag="tr")
                        nc.tensor.transpose(
                            tr[:],
                            comb[:, qq * P : (qq + 1) * P],
                            ident[:DV, :DV],
                        )
                        rec = out_pool.tile([P, 1], f32, tag="rec")
                        nc.vector.reciprocal(rec[:], tr[:, DH : DH + 1])
                        o_t = out_pool.tile([P, DH], f32, tag="o")
                        nc.vector.tensor_scalar_mul(o_t[:], tr[:, :DH], rec[:])
                        r0 = ib * IB + qq * P
                        nc.sync.dma_start(o_d[h, r0 : r0 + P, :], o_t[:])

    nc.compile()
    return nc


# ----------------------------------------------------------------------------
# entry point
# ----------------------------------------------------------------------------

def kernel(hidden_states, q, k, v, bboxes, is_cross, ith, num_heads):
    global LAST_RESULTS
    if is_cross:
        return np.asarray(hidden_states)

    from concourse.bass_utils import run_bass_kernel_spmd

    q = np.ascontiguousarray(np.asarray(q, dtype=np.float32))
    k = np.ascontiguousarray(np.asarray(k, dtype=np.float32))
    v = np.ascontiguousarray(np.asarray(v, dtype=np.float32))
    bboxes = np.asarray(bboxes, dtype=np.float32)
    num_heads = int(num_heads)

    bh, n, dh = q.shape
    assert dh == DH and bh % N_CORES == 0 and n % IB == 0
    heads_per_core = bh // N_CORES
    batch = bh // num_heads
    scale = float(1.0 / np.sqrt(np.float32(dh)))

    perm, padded, starts, n_pad, wA, wB = _group_layout(bboxes, n)
    segments = _chunk_segments(padded, starts, n_pad)
    present_groups = sorted({g for segs in segments for (_, _, g) in segs})

    key = (n, n_pad, heads_per_core, tuple(tuple(s) for s in segments))
    if key not in _PROGRAM_CACHE:
        _PROGRAM_CACHE[key] = _build_program(
            n, n_pad, heads_per_core, segments, present_groups, scale
        )
    nc = _PROGRAM_CACHE[key]

    # host-side input prep
    sel = perm >= 0
    kp = np.zeros((bh, n_pad, dh), np.float32)
    kp[:, sel, :] = k[:, perm[sel], :]
    vt = np.zeros((bh, n_pad, DV), np.float32)
    vt[:, sel, :dh] = v[:, perm[sel], :]
    vt[:, sel, dh] = 1.0
    kT = np.ascontiguousarray(kp.transpose(0, 2, 1))  # [bh, dh, n_pad]
    qT = np.ascontiguousarray(q.transpose(0, 2, 1))  # [bh, dh, n]
    wab = np.ascontiguousarray(np.broadcast_to(wA[None, :], (DV, n)))
    wbb = np.ascontiguousarray(np.broadcast_to(wB[None, :], (DV, n)))
    ident = np.eye(P, dtype=np.float32)

    in_maps = []
    for c in range(N_CORES):
        sl = slice(c * heads_per_core, (c + 1) * heads_per_core)
        in_maps.append({
            "qT": qT[sl], "kT": kT[sl], "vt": vt[sl],
            "wab": wab, "wbb": wbb, "ident": ident,
        })

    trace = bool(int(os.environ.get("BASS_ATTN_TRACE", "0")))
    kwargs = {}
    if trace:
        kwargs = dict(trace=True, trace_cores=list(range(N_CORES)))
    res = run_bass_kernel_spmd(nc, in_maps, core_ids=list(range(N_CORES)), **kwargs)
    LAST_RESULTS = res

    out = np.empty((batch, n, num_heads * dh), np.float32)
    for bh_idx in range(bh):
        c, hh = divmod(bh_idx, heads_per_core)
        b, hd = divmod(bh_idx, num_heads)
        out[b, :, hd * dh : (hd + 1) * dh] = res.results[c]["o"][hh]
    return out


# revision 9
# speedup vs baseline: 2.6537x; 2.6537x over previous
"""Masked multi-head self-attention (sparse_attention) on 8 Trainium2 cores.

Strategy
--------
Shard the fused (batch*heads)=16 leading dim of q/k/v across 8 cores, 2 heads
per core.  Per head the kernel computes S^T = K @ Q^T in [j, i] orientation
(128-row j-chunks on partitions, 512-col i-blocks on the free dim), applies
exp on the scalar engine (no max-subtraction needed: |s*scale| <= ~7 so exp
cannot overflow in fp32, and blocked entries are handled structurally, not
additively), then accumulates O^T = V~^T @ P^T on the tensor engine where
V~ = [V | 1] so the softmax denominators fall out of the same matmuls.

The bbox mask has rank-structure: blocked(i,j) <=> (i in A-only and j in
B-only) or vice versa, where A/B are the two subject boxes.  The host sorts
the j (key/value) axis into [A-only | B-only | rest] with 64-aligned zero
padding, so every 64-row half-chunk belongs to one group.  PV matmuls
accumulate into one PSUM accumulator per group; the final combine applies the
per-i 0/1 weights (wA, wB) and sums the three accumulators — the mask costs
no elementwise work on the n*n tiles at all.  Finally each [81, 512] combined
block is PE-transposed back to [128(i), 81], normalized by the sums column,
and DMA'd out in natural i order (the i/query axis is never permuted).
"""

import math
import os

import numpy as np

N_CORES = 8
P = 128  # partitions / j-chunk rows
IB = 512  # i-block width (psum bank, fp32)
DH = 80  # head dim
DV = DH + 1  # V plus the ones column

_PROGRAM_CACHE = {}
LAST_RESULTS = None  # BassKernelResults of the most recent run (for test.py)


# ----------------------------------------------------------------------------
# host-side mask analysis (mirrors reference._subject_masks / _self_mask)
# ----------------------------------------------------------------------------

def _subject_masks_np(bboxes: np.ndarray, resolution: int) -> np.ndarray:
    b = bboxes[0].astype(np.float32)  # [s, 4]
    x0 = np.round(b[:, 0] * resolution)
    y0 = np.round(b[:, 1] * resolution)
    x1 = np.round(b[:, 2] * resolution)
    y1 = np.round(b[:, 3] * resolution)
    coords = np.arange(resolution, dtype=np.float32)
    xm = (coords[None, :] >= x0[:, None]) & (coords[None, :] < x1[:, None])
    ym = (coords[None, :] >= y0[:, None]) & (coords[None, :] < y1[:, None])
    return (ym[:, :, None] & xm[:, None, :]).reshape(b.shape[0], -1)  # [s, n]


def _group_layout(bboxes: np.ndarray, n: int):
    """Sort the j axis into [A-only | B-only | rest], 64-aligned groups.

    Returns (perm, seg_sizes, group_starts, n_pad, wA, wB) where perm is the
    source index for each padded slot (-1 for zero padding), wA/wB are the
    per-original-i {0,1} combine weights for the A/B accumulators.
    """
    res = int(math.isqrt(n))
    assert res * res == n
    subj = _subject_masks_np(bboxes, res)
    assert subj.shape[0] == 2, "kernel specialized for 2 subject boxes"
    m0, m1 = subj[0], subj[1]
    e0 = m0 & ~m1  # A-only
    e1 = m1 & ~m0  # B-only
    rest = ~(e0 | e1)

    idx = np.arange(n)
    groups = [idx[e0], idx[e1], idx[rest]]

    def ceil64(x):
        return ((x + 63) // 64) * 64

    padded = [ceil64(len(g)) for g in groups]
    n_pad = sum(padded)
    if n_pad % P:
        padded[2] += 64
        n_pad += 64
    perm = np.full(n_pad, -1, dtype=np.int64)
    starts = []
    pos = 0
    for g, plen in zip(groups, padded):
        starts.append(pos)
        perm[pos : pos + len(g)] = g
        pos += plen

    wA = (~e1).astype(np.float32)  # zero the A accumulator for i in B-only
    wB = (~e0).astype(np.float32)  # zero the B accumulator for i in A-only
    return perm, padded, starts, n_pad, wA, wB


def _chunk_segments(padded, starts, n_pad):
    """Per 128-chunk: list of (row_lo, row_hi, group_id) 64-aligned segments."""
    half_group = np.empty(n_pad // 64, dtype=np.int64)
    for gid, (st, plen) in enumerate(zip(starts, padded)):
        half_group[st // 64 : (st + plen) // 64] = gid
    segments = []
    for c in range(n_pad // P):
        g0 = int(half_group[2 * c])
        g1 = int(half_group[2 * c + 1])
        if g0 == g1:
            segments.append([(0, P, g0)])
        else:
            segments.append([(0, 64, g0), (64, P, g1)])
    return segments


# ----------------------------------------------------------------------------
# device program
# ----------------------------------------------------------------------------

def _build_program(n, n_pad, heads_per_core, segments, present_groups, scale):
    import concourse.mybir as mybir
    import concourse.tile as tile
    from concourse import bacc

    f32 = mybir.dt.float32
    f32r = mybir.dt.float32r
    nch = n_pad // P
    n_ib = n // IB
    Exp = mybir.ActivationFunctionType.Exp
    MUL = mybir.AluOpType.mult
    ADD = mybir.AluOpType.add

    nc = bacc.Bacc("TRN2", target_bir_lowering=False, debug=False,
                   num_devices=N_CORES)
    qT_d = nc.dram_tensor("qT", [heads_per_core, DH, n], f32r, kind="ExternalInput")
    kT_d = nc.dram_tensor("kT", [heads_per_core, DH, n_pad], f32r,
                          kind="ExternalInput")
    vt_d = nc.dram_tensor("vt", [heads_per_core, n_pad, DV], f32r,
                          kind="ExternalInput")
    wab_d = nc.dram_tensor("wab", [DV, n], f32, kind="ExternalInput")
    wbb_d = nc.dram_tensor("wbb", [DV, n], f32, kind="ExternalInput")
    id_d = nc.dram_tensor("ident", [P, P], f32, kind="ExternalInput")
    o_d = nc.dram_tensor("o", [heads_per_core, n, DH], f32, kind="ExternalOutput")

    # chunk pairs for batched exp reads
    pairs = [tuple(range(t, min(t + 2, nch))) for t in range(0, nch, 2)]

    # first/last (chunk, row) PV matmul per accumulator group, for start/stop
    first_seg = {}
    last_seg = {}
    for c, segs in enumerate(segments):
        for (r0, _, g) in segs:
            first_seg.setdefault(g, (c, r0))
            last_seg[g] = (c, r0)

    with tile.TileContext(nc) as tc:
        with (
            tc.tile_pool(name="const", bufs=1) as const_pool,
            tc.tile_pool(name="head", bufs=2) as head_pool,
            tc.tile_pool(name="p", bufs=3) as p_pool,
            tc.tile_pool(name="comb", bufs=2) as comb_pool,
            tc.tile_pool(name="out", bufs=4) as out_pool,
            tc.tile_pool(name="s_ps", bufs=2, space="PSUM") as s_pool,
            tc.tile_pool(name="acc_ps", bufs=1, space="PSUM") as acc_pool,
            tc.tile_pool(name="tr_ps", bufs=1, space="PSUM") as tr_pool,
        ):
            ident = const_pool.tile([P, P], f32)
            nc.sync.dma_start(ident[:], id_d[:])
            wab_t = const_pool.tile([DV, n], f32)
            nc.sync.dma_start(wab_t[:], wab_d[:])
            wbb_t = const_pool.tile([DV, n], f32)
            nc.sync.dma_start(wbb_t[:], wbb_d[:])

            for h in range(heads_per_core):
                kT_t = head_pool.tile([DH, nch, P], f32r, tag="kT")
                nc.sync.dma_start(
                    kT_t[:], kT_d[h].rearrange("d (c j) -> d c j", j=P)
                )
                qT_t = head_pool.tile([DH, n], f32r, tag="qT")
                nc.sync.dma_start(qT_t[:], qT_d[h])
                vt_t = head_pool.tile([P, nch, DV], f32r, tag="vt")
                nc.sync.dma_start(
                    vt_t[:], vt_d[h].rearrange("(c p) d -> p c d", p=P)
                )

                for ib in range(n_ib):
                    accs = {
                        g: acc_pool.tile([DV, IB], f32, tag=f"acc{g}",
                                         name=f"acc{g}_{h}_{ib}")
                        for g in present_groups
                    }
                    q_sl = qT_t[:, ib * IB : (ib + 1) * IB]
                    for pr in pairs:
                        s_t = s_pool.tile([P, IB * len(pr)], f32, tag="s")
                        for pi, c in enumerate(pr):
                            nc.tensor.matmul(
                                s_t[:, pi * IB : (pi + 1) * IB],
                                lhsT=kT_t[:, c, :],
                                rhs=q_sl,
                                start=True,
                                stop=True,
                            )
                        p_t = p_pool.tile([P, IB * len(pr)], f32r, tag="p")
                        nc.scalar.activation(p_t[:], s_t[:], Exp, scale=scale)
                        for pi, c in enumerate(pr):
                            for (r0, r1, g) in segments[c]:
                                nc.tensor.matmul(
                                    accs[g][:],
                                    lhsT=vt_t[r0:r1, c, :],
                                    rhs=p_t[r0:r1, pi * IB : (pi + 1) * IB],
                                    start=((c, r0) == first_seg[g]),
                                    stop=((c, r0) == last_seg[g]),
                                )

                    # combine accumulators -> [DV, IB] in SBUF
                    comb = comb_pool.tile([DV, IB], f32, tag="comb")
                    i_sl = slice(ib * IB, (ib + 1) * IB)
                    if 0 in accs and 1 in accs:
                        t1 = comb_pool.tile([DV, IB], f32, tag="t1")
                        nc.vector.tensor_tensor(
                            t1[:], accs[0][:], wab_t[:, i_sl], op=MUL
                        )
                        t2 = comb_pool.tile([DV, IB], f32, tag="t2")
                        nc.vector.tensor_tensor(
                            t2[:], accs[1][:], wbb_t[:, i_sl], op=MUL
                        )
                        nc.vector.tensor_tensor(t1[:], t1[:], t2[:], op=ADD)
                        nc.vector.tensor_tensor(comb[:], t1[:], accs[2][:], op=ADD)
                    elif 0 in accs or 1 in accs:
                        g = 0 if 0 in accs else 1
                        w_t = wab_t if g == 0 else wbb_t
                        t1 = comb_pool.tile([DV, IB], f32, tag="t1")
                        nc.vector.tensor_tensor(
                            t1[:], accs[g][:], w_t[:, i_sl], op=MUL
                        )
                        nc.vector.tensor_tensor(comb[:], t1[:], accs[2][:], op=ADD)
                    else:
                        nc.vector.tensor_copy(comb[:], accs[2][:])

                    # transpose quarters, normalize, store
                    for qq in range(IB // P):
                        tr = tr_pool.tile([P, DV], f32, tag="tr")
                        nc.tensor.transpose(
                            tr[:],
                            comb[:, qq * P : (qq + 1) * P],
                            ident[:DV, :DV],
                        )
                        rec = out_pool.tile([P, 1], f32, tag="rec")
                        nc.vector.reciprocal(rec[:], tr[:, DH : DH + 1])
                        o_t = out_pool.tile([P, DH], f32, tag="o")
                        nc.vector.tensor_scalar_mul(o_t[:], tr[:, :DH], rec[:])
                        r0 = ib * IB + qq * P
                        nc.sync.dma_start(o_d[h, r0 : r0 + P, :], o_t[:])

    nc.compile()
    return nc


# ----------------------------------------------------------------------------
# entry point
# ----------------------------------------------------------------------------

def kernel(hidden_states, q, k, v, bboxes, is_cross, ith, num_heads):
    global LAST_RESULTS
    if is_cross:
        return np.asarray(hidden_states)

    from concourse.bass_utils import run_bass_kernel_spmd

    q = np.ascontiguousarray(np.asarray(q, dtype=np.float32))
    k = np.ascontiguousarray(np.asarray(k, dtype=np.float32))
    v = np.ascontiguousarray(np.asarray(v, dtype=np.float32))
    bboxes = np.asarray(bboxes, dtype=np.float32)
    num_heads = int(num_heads)

    bh, n, dh = q.shape
    assert dh == DH and bh % N_CORES == 0 and n % IB == 0
    heads_per_core = bh // N_CORES
    batch = bh // num_heads
    scale = float(1.0 / np.sqrt(np.float32(dh)))

    perm, padded, starts, n_pad, wA, wB = _group_layout(bboxes, n)
    segments = _chunk_segments(padded, starts, n_pad)
    present_groups = sorted({g for segs in segments for (_, _, g) in segs})

    key = (n, n_pad, heads_per_core, tuple(tuple(s) for s in segments))
    if key not in _PROGRAM_CACHE:
        _PROGRAM_CACHE[key] = _build_program(
            n, n_pad, heads_per_core, segments, present_groups, scale
        )
    nc = _PROGRAM_CACHE[key]

    # host-side input prep
    sel = perm >= 0
    kp = np.zeros((bh, n_pad, dh), np.float32)
    kp[:, sel, :] = k[:, perm[sel], :]
    vt = np.zeros((bh, n_pad, DV), np.float32)
    vt[:, sel, :dh] = v[:, perm[sel], :]
    vt[:, sel, dh] = 1.0
    kT = np.ascontiguousarray(kp.transpose(0, 2, 1))  # [bh, dh, n_pad]
    qT = np.ascontiguousarray(q.transpose(0, 2, 1))  # [bh, dh, n]
    wab = np.ascontiguousarray(np.broadcast_to(wA[None, :], (DV, n)))
    wbb = np.ascontiguousarray(np.broadcast_to(wB[None, :], (DV, n)))
    ident = np.eye(P, dtype=np.float32)

    in_maps = []
    for c in range(N_CORES):
        sl = slice(c * heads_per_core, (c + 1) * heads_per_core)
        in_maps.append({
            "qT": qT[sl], "kT": kT[sl], "vt": vt[sl],
            "wab": wab, "wbb": wbb, "ident": ident,
        })

    trace = bool(int(os.environ.get("BASS_ATTN_TRACE", "0")))
    kwargs = {}
    if trace:
        kwargs = dict(trace=True, trace_cores=list(range(N_CORES)))
    res = run_bass_kernel_spmd(nc, in_maps, core_ids=list(range(N_CORES)), **kwargs)
    LAST_RESULTS = res

    out = np.empty((batch, n, num_heads * dh), np.float32)
    for bh_idx in range(bh):
        c, hh = divmod(bh_idx, heads_per_core)
        b, hd = divmod(bh_idx, num_heads)
        out[b, :, hd * dh : (hd + 1) * dh] = res.results[c]["o"][hh]
    return out


# revision 12
# speedup vs baseline: 2.8185x; 1.0621x over previous
"""Masked multi-head self-attention (sparse_attention) on 8 Trainium2 cores.

Strategy
--------
Shard the fused (batch*heads)=16 leading dim of q/k/v across 8 cores, 2 heads
per core.  Per head the kernel computes S^T = K @ Q^T in [j, i] orientation
(128-row j-chunks on partitions, 512-col i-blocks on the free dim), applies
exp on the scalar engine (no max-subtraction needed: |s*scale| <= ~7 so exp
cannot overflow in fp32, and blocked entries are handled structurally, not
additively), then accumulates O^T = V~^T @ P^T on the tensor engine where
V~ = [V | 1] so the softmax denominators fall out of the same matmuls.

The bbox mask has rank-structure: blocked(i,j) <=> (i in A-only and j in
B-only) or vice versa, where A/B are the two subject boxes.  The host sorts
the j (key/value) axis into [A-only | B-only | rest] with 64-aligned zero
padding, so every 64-row half-chunk belongs to one group.  PV matmuls
accumulate into one PSUM accumulator per group; the final combine applies the
per-i 0/1 weights (wA, wB) and sums the three accumulators — the mask costs
no elementwise work on the n*n tiles at all.  Finally each [81, 512] combined
block is PE-transposed back to [128(i), 81], normalized by the sums column,
and DMA'd out in natural i order (the i/query axis is never permuted).
"""

import math
import os

import numpy as np

N_CORES = 8
P = 128  # partitions / j-chunk rows
IB = 512  # i-block width (psum bank, fp32)
DH = 80  # head dim
DV = DH + 1  # V plus the ones column

_PROGRAM_CACHE = {}
LAST_RESULTS = None  # BassKernelResults of the most recent run (for test.py)


# ----------------------------------------------------------------------------
# host-side mask analysis (mirrors reference._subject_masks / _self_mask)
# ----------------------------------------------------------------------------

def _subject_masks_np(bboxes: np.ndarray, resolution: int) -> np.ndarray:
    b = bboxes[0].astype(np.float32)  # [s, 4]
    x0 = np.round(b[:, 0] * resolution)
    y0 = np.round(b[:, 1] * resolution)
    x1 = np.round(b[:, 2] * resolution)
    y1 = np.round(b[:, 3] * resolution)
    coords = np.arange(resolution, dtype=np.float32)
    xm = (coords[None, :] >= x0[:, None]) & (coords[None, :] < x1[:, None])
    ym = (coords[None, :] >= y0[:, None]) & (coords[None, :] < y1[:, None])
    return (ym[:, :, None] & xm[:, None, :]).reshape(b.shape[0], -1)  # [s, n]


def _group_layout(bboxes: np.ndarray, n: int):
    """Sort the j axis into [A-only | B-only | rest], 64-aligned groups.

    Returns (perm, seg_sizes, group_starts, n_pad, wA, wB) where perm is the
    source index for each padded slot (-1 for zero padding), wA/wB are the
    per-original-i {0,1} combine weights for the A/B accumulators.
    """
    res = int(math.isqrt(n))
    assert res * res == n
    subj = _subject_masks_np(bboxes, res)
    assert subj.shape[0] == 2, "kernel specialized for 2 subject boxes"
    m0, m1 = subj[0], subj[1]
    e0 = m0 & ~m1  # A-only
    e1 = m1 & ~m0  # B-only
    rest = ~(e0 | e1)

    idx = np.arange(n)
    groups = [idx[e0], idx[e1], idx[rest]]

    def ceil64(x):
        return ((x + 63) // 64) * 64

    padded = [ceil64(len(g)) for g in groups]
    n_pad = sum(padded)
    if n_pad % P:
        padded[2] += 64
        n_pad += 64
    perm = np.full(n_pad, -1, dtype=np.int64)
    starts = []
    pos = 0
    for g, plen in zip(groups, padded):
        starts.append(pos)
        perm[pos : pos + len(g)] = g
        pos += plen

    wA = (~e1).astype(np.float32)  # zero the A accumulator for i in B-only
    wB = (~e0).astype(np.float32)  # zero the B accumulator for i in A-only
    return perm, padded, starts, n_pad, wA, wB


def _chunk_segments(padded, starts, n_pad):
    """Per 128-chunk: list of (row_lo, row_hi, group_id) 64-aligned segments."""
    half_group = np.empty(n_pad // 64, dtype=np.int64)
    for gid, (st, plen) in enumerate(zip(starts, padded)):
        half_group[st // 64 : (st + plen) // 64] = gid
    segments = []
    for c in range(n_pad // P):
        g0 = int(half_group[2 * c])
        g1 = int(half_group[2 * c + 1])
        if g0 == g1:
            segments.append([(0, P, g0)])
        else:
            segments.append([(0, 64, g0), (64, P, g1)])
    return segments


# ----------------------------------------------------------------------------
# device program
# ----------------------------------------------------------------------------

def _build_program(n, n_pad, heads_per_core, segments, present_groups, scale):
    import concourse.mybir as mybir
    import concourse.tile as tile
    from concourse import bacc

    f32 = mybir.dt.float32
    f32r = mybir.dt.float32r
    nch = n_pad // P
    n_ib = n // IB
    Exp = mybir.ActivationFunctionType.Exp
    MUL = mybir.AluOpType.mult
    ADD = mybir.AluOpType.add

    nc = bacc.Bacc("TRN2", target_bir_lowering=False, debug=False,
                   num_devices=N_CORES)
    qT_d = nc.dram_tensor("qT", [heads_per_core, DH, n], f32r, kind="ExternalInput")
    kT_d = nc.dram_tensor("kT", [heads_per_core, DH, n_pad], f32r,
                          kind="ExternalInput")
    vt_d = nc.dram_tensor("vt", [heads_per_core, n_pad, DV], f32r,
                          kind="ExternalInput")
    wab_d = nc.dram_tensor("wab", [DV, n], f32, kind="ExternalInput")
    wbb_d = nc.dram_tensor("wbb", [DV, n], f32, kind="ExternalInput")
    id_d = nc.dram_tensor("ident", [P, P], f32, kind="ExternalInput")
    o_d = nc.dram_tensor("o", [heads_per_core, n, DH], f32, kind="ExternalOutput")

    # chunk pairs for batched exp reads
    pairs = [tuple(range(t, min(t + 2, nch))) for t in range(0, nch, 2)]

    # first/last (chunk, row) PV matmul per accumulator group, for start/stop
    first_seg = {}
    last_seg = {}
    for c, segs in enumerate(segments):
        for (r0, _, g) in segs:
            first_seg.setdefault(g, (c, r0))
            last_seg[g] = (c, r0)

    with tile.TileContext(nc) as tc:
        with (
            tc.tile_pool(name="const", bufs=1) as const_pool,
            tc.tile_pool(name="head", bufs=2) as head_pool,
            tc.tile_pool(name="p", bufs=3) as p_pool,
            tc.tile_pool(name="comb", bufs=2) as comb_pool,
            tc.tile_pool(name="out", bufs=4) as out_pool,
            tc.tile_pool(name="s_ps", bufs=2, space="PSUM") as s_pool,
            tc.tile_pool(name="acc_ps", bufs=1, space="PSUM") as acc_pool,
            tc.tile_pool(name="tr_ps", bufs=1, space="PSUM") as tr_pool,
        ):
            ident = const_pool.tile([P, P], f32)
            wab_t = const_pool.tile([DV, n], f32)
            wbb_t = const_pool.tile([DV, n], f32)

            def load_head(h):
                kT_t = head_pool.tile([DH, nch, P], f32r, tag="kT",
                                      name=f"kT_{h}")
                qT_t = head_pool.tile([DH, n], f32r, tag="qT", name=f"qT_{h}")
                vt_t = head_pool.tile([P, nch, DV], f32r, tag="vt",
                                      name=f"vt_{h}")
                kT_src = kT_d[h].rearrange("d (c j) -> d c j", j=P)
                vt_src = vt_d[h].rearrange("(c p) d -> p c d", p=P)
                # sliced loads, first-needed slices first, so chunk-0 compute
                # starts long before the full head is resident
                cuts = [0, 2, 6, 12, 20, nch]
                nc.sync.dma_start(kT_t[:, 0:2, :], kT_src[:, 0:2, :])
                nc.sync.dma_start(qT_t[:, 0:IB], qT_d[h][:, 0:IB])
                nc.sync.dma_start(vt_t[:, 0:2, :], vt_src[:, 0:2, :])
                for c0, c1 in zip(cuts[1:], cuts[2:]):
                    nc.sync.dma_start(kT_t[:, c0:c1, :], kT_src[:, c0:c1, :])
                    nc.sync.dma_start(vt_t[:, c0:c1, :], vt_src[:, c0:c1, :])
                nc.sync.dma_start(qT_t[:, IB:], qT_d[h][:, IB:])
                return kT_t, qT_t, vt_t

            head_tiles = {0: load_head(0)}
            nc.sync.dma_start(ident[:], id_d[:])
            nc.sync.dma_start(wab_t[:], wab_d[:])
            nc.sync.dma_start(wbb_t[:], wbb_d[:])

            pending_epilogue = None

            for h in range(heads_per_core):
                kT_t, qT_t, vt_t = head_tiles[h]
                if h + 1 < heads_per_core:
                    head_tiles[h + 1] = load_head(h + 1)

                for ib in range(n_ib):
                    accs = {
                        g: acc_pool.tile([DV, IB], f32, tag=f"acc{g}",
                                         name=f"acc{g}_{h}_{ib}")
                        for g in present_groups
                    }
                    q_sl = qT_t[:, ib * IB : (ib + 1) * IB]
                    for t, pr in enumerate(pairs):
                        if t == 2 and pending_epilogue is not None:
                            pending_epilogue()
                            pending_epilogue = None
                        s_t = s_pool.tile([P, IB * len(pr)], f32, tag="s")
                        for pi, c in enumerate(pr):
                            nc.tensor.matmul(
                                s_t[:, pi * IB : (pi + 1) * IB],
                                lhsT=kT_t[:, c, :],
                                rhs=q_sl,
                                start=True,
                                stop=True,
                            )
                        p_t = p_pool.tile([P, IB * len(pr)], f32r, tag="p")
                        nc.scalar.activation(p_t[:], s_t[:], Exp, scale=scale)
                        for pi, c in enumerate(pr):
                            for (r0, r1, g) in segments[c]:
                                nc.tensor.matmul(
                                    accs[g][:],
                                    lhsT=vt_t[r0:r1, c, :],
                                    rhs=p_t[r0:r1, pi * IB : (pi + 1) * IB],
                                    start=((c, r0) == first_seg[g]),
                                    stop=((c, r0) == last_seg[g]),
                                )

                    def make_epilogue(accs=accs, h=h, ib=ib):
                        def epilogue():
                            # combine accumulators -> [DV, IB] in SBUF
                            comb = comb_pool.tile([DV, IB], f32, tag="comb",
                                                  name=f"comb_{h}_{ib}")
                            i_sl = slice(ib * IB, (ib + 1) * IB)
                            if 0 in accs and 1 in accs:
                                t1 = comb_pool.tile([DV, IB], f32, tag="t1",
                                                    name=f"t1_{h}_{ib}")
                                nc.vector.tensor_tensor(
                                    t1[:], accs[0][:], wab_t[:, i_sl], op=MUL
                                )
                                t2 = comb_pool.tile([DV, IB], f32, tag="t2",
                                                    name=f"t2_{h}_{ib}")
                                nc.vector.tensor_tensor(
                                    t2[:], accs[1][:], wbb_t[:, i_sl], op=MUL
                                )
                                nc.vector.tensor_tensor(t1[:], t1[:], t2[:],
                                                        op=ADD)
                                nc.vector.tensor_tensor(comb[:], t1[:],
                                                        accs[2][:], op=ADD)
                            elif 0 in accs or 1 in accs:
                                g = 0 if 0 in accs else 1
                                w_t = wab_t if g == 0 else wbb_t
                                t1 = comb_pool.tile([DV, IB], f32, tag="t1",
                                                    name=f"t1_{h}_{ib}")
                                nc.vector.tensor_tensor(
                                    t1[:], accs[g][:], w_t[:, i_sl], op=MUL
                                )
                                nc.vector.tensor_tensor(comb[:], t1[:],
                                                        accs[2][:], op=ADD)
                            else:
                                nc.vector.tensor_copy(comb[:], accs[2][:])

                            # transpose quarters, normalize, store
                            for qq in range(IB // P):
                                tr = tr_pool.tile([P, DV], f32, tag="tr",
                                                  name=f"tr_{h}_{ib}_{qq}")
                                nc.tensor.transpose(
                                    tr[:],
                                    comb[:, qq * P : (qq + 1) * P],
                                    ident[:DV, :DV],
                                )
                                rec = out_pool.tile([P, 1], f32, tag="rec",
                                                    name=f"rec_{h}_{ib}_{qq}")
                                nc.vector.reciprocal(rec[:], tr[:, DH : DH + 1])
                                o_t = out_pool.tile([P, DH], f32, tag="o",
                                                    name=f"o_{h}_{ib}_{qq}")
                                nc.vector.tensor_scalar_mul(o_t[:], tr[:, :DH],
                                                            rec[:])
                                r0 = ib * IB + qq * P
                                nc.sync.dma_start(o_d[h, r0 : r0 + P, :], o_t[:])
                        return epilogue

                    pending_epilogue = make_epilogue()

            if pending_epilogue is not None:
                pending_epilogue()

    nc.compile()
    return nc


# ----------------------------------------------------------------------------
# entry point
# ----------------------------------------------------------------------------

def kernel(hidden_states, q, k, v, bboxes, is_cross, ith, num_heads):
    global LAST_RESULTS
    if is_cross:
        return np.asarray(hidden_states)

    from concourse.bass_utils import run_bass_kernel_spmd

    q = np.ascontiguousarray(np.asarray(q, dtype=np.float32))
    k = np.ascontiguousarray(np.asarray(k, dtype=np.float32))
    v = np.ascontiguousarray(np.asarray(v, dtype=np.float32))
    bboxes = np.asarray(bboxes, dtype=np.float32)
    num_heads = int(num_heads)

    bh, n, dh = q.shape
    assert dh == DH and bh % N_CORES == 0 and n % IB == 0
    heads_per_core = bh // N_CORES
    batch = bh // num_heads
    scale = float(1.0 / np.sqrt(np.float32(dh)))

    perm, padded, starts, n_pad, wA, wB = _group_layout(bboxes, n)
    segments = _chunk_segments(padded, starts, n_pad)
    present_groups = sorted({g for segs in segments for (_, _, g) in segs})

    key = (n, n_pad, heads_per_core, tuple(tuple(s) for s in segments))
    if key not in _PROGRAM_CACHE:
        _PROGRAM_CACHE[key] = _build_program(
            n, n_pad, heads_per_core, segments, present_groups, scale
        )
    nc = _PROGRAM_CACHE[key]

    # host-side input prep
    sel = perm >= 0
    kp = np.zeros((bh, n_pad, dh), np.float32)
    kp[:, sel, :] = k[:, perm[sel], :]
    vt = np.zeros((bh, n_pad, DV), np.float32)
    vt[:, sel, :dh] = v[:, perm[sel], :]
    vt[:, sel, dh] = 1.0
    kT = np.ascontiguousarray(kp.transpose(0, 2, 1))  # [bh, dh, n_pad]
    qT = np.ascontiguousarray(q.transpose(0, 2, 1))  # [bh, dh, n]
    wab = np.ascontiguousarray(np.broadcast_to(wA[None, :], (DV, n)))
    wbb = np.ascontiguousarray(np.broadcast_to(wB[None, :], (DV, n)))
    ident = np.eye(P, dtype=np.float32)

    in_maps = []
    for c in range(N_CORES):
        sl = slice(c * heads_per_core, (c + 1) * heads_per_core)
        in_maps.append({
            "qT": qT[sl], "kT": kT[sl], "vt": vt[sl],
            "wab": wab, "wbb": wbb, "ident": ident,
        })

    trace = bool(int(os.environ.get("BASS_ATTN_TRACE", "0")))
    kwargs = {}
    if trace:
        kwargs = dict(trace=True, trace_cores=list(range(N_CORES)))
    res = run_bass_kernel_spmd(nc, in_maps, core_ids=list(range(N_CORES)), **kwargs)
    LAST_RESULTS = res

    out = np.empty((batch, n, num_heads * dh), np.float32)
    for bh_idx in range(bh):
        c, hh = divmod(bh_idx, heads_per_core)
        b, hd = divmod(bh_idx, num_heads)
        out[b, :, hd * dh : (hd + 1) * dh] = res.results[c]["o"][hh]
    return out
